# revision 10
# baseline (speedup 1.0000x reference)
"""minLSTM (2-layer, B=4, S=4096, D=1024) on 8 Trainium2 NeuronCores.

Sharding: core k -> (batch b = k//2, channel half h = k%2).
Each core computes all 4096 timesteps for its batch and its 512 channels:
  - gates via PE matmuls in bf16 (lhsT = W^T shard, rhs = x^T), laid out
    (gate-channel partition x token free) so the recurrence layout is
    native; bf16 weights enable fast-weight-load and halve DMA traffic,
  - normalized gates f' = sig(f)/(sig(f)+sig(i)) via ACT sigmoids + one
    DVE reciprocal; the cheap elementwise products are offloaded to the
    otherwise-idle GPSIMD engine so the DVE chain stays under the PE time,
  - g = max(z + 0.5, sigmoid(z)) with z = cell + b the biased
    pre-activation (single ACT identity), fused via scalar_tensor_tensor,
  - the recurrence c_t = f'_t c_{t-1} + i'_t g_t with the DVE
    tensor_tensor_scan instruction in linear space (values bounded in
    [~4.5e-5, 10.5], so no log-space machinery is needed),
  - h = sig(o) * c.
Between the two layers, channel-half pairs exchange h1 (bf16) via pairwise
AllGather collectives (one per 512-token block, overlapped with compute).

Self-contained: hardcodes shapes; only imports the system concourse repo.
"""
import sys

if '/opt/trn_rl_repo' not in sys.path:
    sys.path.insert(0, '/opt/trn_rl_repo')

import numpy as np

B, S, D = 4, 4096, 1024
NCORES = 8
HALF = D // 2           # channels per core: 512
NCHUNK = HALF // 128    # 4 partition chunks of 128 channels
NKT = D // 128          # 8 contraction k-tiles
TBLK = 512              # token block
NBLK = S // TBLK        # 8 token blocks
GCH = 4 * HALF          # gate channels per core: 2048

_CACHE = {}

WSCALE = 64.0   # fp8 weight pre-scale (keeps W entries out of e4m3 denormals)


def _split_multi_waits(nc):
    """This walrus build rejects >1 sync wait per instruction. Hoist extra
    waits onto same-engine NoOps inserted just before; engine-queue program
    order makes this semantically identical."""
    from concourse import mybir
    n = 0
    for fn in nc.m.functions:
        for blk in fn.blocks:
            insts = list(blk.instructions)
            new = []
            changed = False
            for inst in insts:
                si = inst.sync_info
                ow = list(si.on_wait) if si is not None and si.on_wait else []
                if len(ow) > 1:
                    changed = True
                    for w in ow[:-1]:
                        n += 1
                        nop = mybir.InstNoOp(name=f"I-wsplit-{n}", ins=[], outs=[])
                        nop.engine = inst.engine
                        nop.sync_info = mybir.SyncInfo(on_wait=[w], on_update=[])
                        new.append(nop)
                    si.on_wait = [ow[-1]]
                new.append(inst)
            if changed:
                blk.instructions = new
    return n


def _build_nc(mm_mode="bf16", sim_local=False):
    import concourse.bass as bass
    import concourse.mybir as mybir
    import concourse.tile as tile

    f32 = mybir.dt.float32
    fmm = {"f32r": mybir.dt.float32r, "f32": f32,
           "bf16": mybir.dt.bfloat16, "fp8": mybir.dt.float8e4}[mm_mode]
    fh1 = {"f32r": f32, "f32": f32,
           "bf16": mybir.dt.bfloat16, "fp8": mybir.dt.float8e4}[mm_mode]
    is_fp8 = mm_mode == "fp8"
    gsc = 1.0 / WSCALE if is_fp8 else 1.0   # undo weight pre-scale at ACT
    AF = mybir.ActivationFunctionType
    ALU = mybir.AluOpType
    DR = mybir.MatmulPerfMode.DoubleRow

    nc = bass.Bass("TRN2", target_bir_lowering=False, debug=False,
                   num_devices=NCORES)

    xT_d = nc.dram_tensor("xT", [D, S], fmm, kind="ExternalInput").ap()
    w_d = [nc.dram_tensor(f"w{l}t", [D, GCH], fmm, kind="ExternalInput").ap()
           for l in range(2)]
    ba_d = [nc.dram_tensor(f"b{l}a", [128, 16], f32, kind="ExternalInput").ap()
            for l in range(2)]
    cp_d = [nc.dram_tensor(f"cp{l}", [128, 4], f32, kind="ExternalInput").ap()
            for l in range(2)]
    h2t_d = nc.dram_tensor("h2t", [HALF, S], f32, kind="ExternalOutput").ap()

    with tile.TileContext(nc) as tc:
        with tc.tile_pool(name="wp", bufs=1) as wp, \
             tc.tile_pool(name="xkp", bufs=2) as xkp, \
             tc.tile_pool(name="gp2", bufs=2) as gp2, \
             tc.tile_pool(name="gp3", bufs=3) as gp3, \
             tc.tile_pool(name="cp", bufs=1) as cpool, \
             tc.tile_pool(name="psum", bufs=8, space="PSUM") as psum, \
             tc.tile_pool(name="dstage", bufs=2, space="DRAM") as dstage, \
             tc.tile_pool(name="dfull", bufs=8, space="DRAM") as dfull:

            # h1 gathered blocks must persist through layer 2: 8 live tiles
            h1f = [dfull.tile([D, TBLK], fh1, tag="h1f", name=f"h1f{t}")
                   for t in range(NBLK)]

            for l in range(2):
                w_ks = []
                if is_fp8:
                    for k in range(NKT // 2):
                        wk = wp.tile([128, 2, GCH], fmm, tag=f"Wk{k}",
                                     name=f"w{l}_{k}")
                        for i in range(2):
                            nc.sync.dma_start(
                                wk[:, i, :],
                                w_d[l][(2 * k + i) * 128:(2 * k + i + 1) * 128, :])
                        w_ks.append(wk)
                else:
                    for k in range(NKT):
                        wk = wp.tile([128, GCH], fmm, tag=f"Wk{k}", name=f"w{l}_{k}")
                        nc.sync.dma_start(wk[:], w_d[l][k * 128:(k + 1) * 128, :])
                        w_ks.append(wk)
                ba = cpool.tile([128, 16], f32, tag=f"ba{l}", name=f"ba{l}")
                nc.sync.dma_start(ba[:], ba_d[l][:])
                cp = cpool.tile([128, 4], f32, tag=f"cp{l}", name=f"cp{l}")
                nc.sync.dma_start(cp[:], cp_d[l][:])

                carry = [None] * NCHUNK
                for t in range(NBLK):
                    xk_ks = []
                    if is_fp8:
                        for k in range(NKT // 2):
                            xkt = xkp.tile([128, 2, TBLK], fmm, tag=f"xk{l}_{k}",
                                           name=f"xk{l}_{t}_{k}")
                            for i in range(2):
                                r0 = (2 * k + i) * 128
                                if l == 0:
                                    src = xT_d[r0:r0 + 128,
                                               t * TBLK:(t + 1) * TBLK]
                                else:
                                    src = h1f[t][r0:r0 + 128, :]
                                nc.sync.dma_start(xkt[:, i, :], src)
                            xk_ks.append(xkt)
                    else:
                        for k in range(NKT):
                            xkt = xkp.tile([128, TBLK], fmm, tag=f"xk{l}_{k}",
                                           name=f"xk{l}_{t}_{k}")
                            if l == 0:
                                src = xT_d[k * 128:(k + 1) * 128,
                                           t * TBLK:(t + 1) * TBLK]
                            else:
                                src = h1f[t][k * 128:(k + 1) * 128, :]
                            nc.sync.dma_start(
                                xkt[:],
                                src if src.dtype == fmm else src.bitcast(fmm))
                            xk_ks.append(xkt)

                    if l == 0:
                        h1own = dstage.tile([HALF, TBLK], fh1, tag="h1own",
                                            name=f"h1own{t}")

                    for j in range(NCHUNK):
                        ps = {}
                        for qi, q in enumerate(("i", "f", "o", "cell")):
                            ct = qi * NCHUNK + j
                            p = psum.tile([128, TBLK], f32, tag="ps",
                                          name=f"ps_{q}{l}_{t}_{j}")
                            if is_fp8:
                                for k in range(NKT // 2):
                                    nc.tensor.matmul(
                                        p[:],
                                        w_ks[k][:, :, ct * 128:(ct + 1) * 128],
                                        xk_ks[k][:],
                                        start=(k == 0), stop=(k == NKT // 2 - 1),
                                        perf_mode=DR)
                            else:
                                for k in range(NKT):
                                    nc.tensor.matmul(
                                        p[:],
                                        w_ks[k][:, ct * 128:(ct + 1) * 128],
                                        xk_ks[k][:],
                                        start=(k == 0), stop=(k == NKT - 1))
                            ps[q] = p

                        sf = gp2.tile([128, TBLK], f32, tag="sf", name=f"sf{l}{t}{j}")
                        nc.scalar.activation(sf[:], ps["f"][:], AF.Sigmoid,
                                             bias=ba[:, 4 + j:5 + j], scale=gsc)
                        si = gp2.tile([128, TBLK], f32, tag="si", name=f"si{l}{t}{j}")
                        nc.scalar.activation(si[:], ps["i"][:], AF.Sigmoid,
                                             bias=ba[:, j:j + 1], scale=gsc)
                        # z = cell + b_cell (pre-activation); sg = sig(z);
                        # g = max(z + 0.5, sg)  == reference's g(z)
                        z = gp2.tile([128, TBLK], f32, tag="z", name=f"z{l}{t}{j}")
                        nc.scalar.activation(z[:], ps["cell"][:], AF.Identity,
                                             bias=ba[:, 12 + j:13 + j], scale=gsc)
                        sg = gp2.tile([128, TBLK], f32, tag="sg", name=f"sg{l}{t}{j}")
                        nc.scalar.activation(sg[:], z[:], AF.Sigmoid)
                        so = gp2.tile([128, TBLK], f32, tag="so", name=f"so{l}{t}{j}")
                        nc.scalar.activation(so[:], ps["o"][:], AF.Sigmoid,
                                             bias=ba[:, 8 + j:9 + j], scale=gsc)

                        ssum = gp2.tile([128, TBLK], f32, tag="ssum",
                                        name=f"ss{l}{t}{j}")
                        nc.vector.tensor_tensor(ssum[:], sf[:], si[:], ALU.add)
                        r = gp2.tile([128, TBLK], f32, tag="r", name=f"r{l}{t}{j}")
                        nc.vector.reciprocal(r[:], ssum[:])
                        # cheap products go to the otherwise-idle GPSIMD
                        a = gp3.tile([128, TBLK], f32, tag="a", name=f"a{l}{t}{j}")
                        nc.gpsimd.tensor_tensor(a[:], sf[:], r[:], ALU.mult)
                        ipr = gp2.tile([128, TBLK], f32, tag="ipr",
                                       name=f"ip{l}{t}{j}")
                        nc.gpsimd.tensor_tensor(ipr[:], si[:], r[:], ALU.mult)

                        g = gp2.tile([128, TBLK], f32, tag="g", name=f"g{l}{t}{j}")
                        nc.vector.scalar_tensor_tensor(
                            g[:], z[:], 0.5, sg[:], ALU.add, ALU.max)
                        bt = gp3.tile([128, TBLK], f32, tag="bt", name=f"bt{l}{t}{j}")
                        nc.gpsimd.tensor_tensor(bt[:], ipr[:], g[:], ALU.mult)
                        c = gp3.tile([128, TBLK], f32, tag=f"c{j}", name=f"c{l}{t}{j}")
                        init = cp[:, j:j + 1] if t == 0 else carry[j]
                        nc.vector.tensor_tensor_scan(c[:], a[:], bt[:], init,
                                                     ALU.mult, ALU.add)
                        carry[j] = c[:, TBLK - 1:TBLK]
                        hdt = fh1 if l == 0 else f32
                        h = gp3.tile([128, TBLK], hdt, tag=f"h{l}", name=f"h{l}{t}{j}")
                        nc.vector.tensor_tensor(h[:], so[:], c[:], ALU.mult)

                        if l == 0:
                            nc.sync.dma_start(
                                h1own[j * 128:(j + 1) * 128, :], h[:])
                        else:
                            nc.sync.dma_start(
                                h2t_d[j * 128:(j + 1) * 128,
                                      t * TBLK:(t + 1) * TBLK], h[:])

                    if l == 0:
                        if sim_local:
                            # collective-free stand-in with the same data
                            # volume, for single-core TimelineSim modeling
                            nc.sync.dma_start(h1f[t][0:HALF, :], h1own[:])
                            nc.sync.dma_start(h1f[t][HALF:D, :], h1own[:])
                        else:
                            nc.gpsimd.collective_compute(
                                "AllGather", ALU.bypass,
                                replica_groups=[[0, 1], [2, 3], [4, 5], [6, 7]],
                                ins=[h1own.opt()],
                                outs=[h1f[t].opt()],
                            )

    _split_multi_waits(nc)
    return nc


def _shard_inputs(x, W0, b0, W1, b1, c0_prev, c1_prev, mm_mode="bf16"):
    import ml_dtypes
    mmdt = {"f32r": np.float32, "f32": np.float32,
            "bf16": ml_dtypes.bfloat16,
            "fp8": ml_dtypes.float8_e4m3fn}[mm_mode]
    wmul = WSCALE if mm_mode == "fp8" else 1.0
    x = np.asarray(x, dtype=np.float32)
    in_maps = []
    xT = [np.ascontiguousarray(x[b].T.astype(mmdt)) for b in range(B)]
    per_layer = []
    for (W, bb) in ((W0, b0), (W1, b1)):
        W = np.asarray(W, dtype=np.float32)
        bb = np.asarray(bb, dtype=np.float32)
        halves = []
        for h in range(2):
            rows = np.concatenate(
                [q * D + h * HALF + np.arange(HALF) for q in range(4)])
            wt = np.ascontiguousarray(
                (W[rows, :].T * np.float32(wmul)).astype(mmdt))  # (D, GCH)
            ba = np.ascontiguousarray(bb[rows].reshape(16, 128).T)  # (128,16)
            halves.append((wt, ba))
        per_layer.append(halves)
    cps = []
    for cprev in (c0_prev, c1_prev):
        cprev = np.asarray(cprev, dtype=np.float32)
        halves = []
        for b in range(B):
            row = []
            for h in range(2):
                seg = cprev[b, 0, h * HALF:(h + 1) * HALF]
                row.append(np.ascontiguousarray(seg.reshape(4, 128).T))
            halves.append(row)
        cps.append(halves)
    for k in range(NCORES):
        b, h = k // 2, k % 2
        m = {"xT": xT[b]}
        for l in range(2):
            wt, ba = per_layer[l][h]
            m[f"w{l}t"] = wt
            m[f"b{l}a"] = ba
            m[f"cp{l}"] = cps[l][b][h]
        in_maps.append(m)
    return in_maps


import os
MM_MODE = os.environ.get("MINLSTM_MM_MODE", "bf16")


def _get_nc():
    if "nc" not in _CACHE:
        _CACHE["nc"] = _build_nc(mm_mode=MM_MODE)
    return _CACHE["nc"]


def kernel(x, W0, b0, W1, b1, c0_prev, c1_prev):
    from concourse.bass_utils import run_bass_kernel_spmd

    nc = _get_nc()
    in_maps = _shard_inputs(x, W0, b0, W1, b1, c0_prev, c1_prev, MM_MODE)
    res = run_bass_kernel_spmd(nc, in_maps, list(range(NCORES)))
    out = np.empty((B, S, D), dtype=np.float32)
    for k in range(NCORES):
        b, h = k // 2, k % 2
        out[b, :, h * HALF:(h + 1) * HALF] = res.results[k]["h2t"].T
    return out


# revision 16
# speedup vs baseline: 1.0216x; 1.0216x over previous
"""minLSTM (2-layer, B=4, S=4096, D=1024) on 8 Trainium2 NeuronCores.

Sharding: core k -> (batch b = k//2, channel half h = k%2).
Each core computes all 4096 timesteps for its batch and its 512 channels:
  - gates via PE matmuls in bf16 (lhsT = W^T shard, rhs = x^T), laid out
    (gate-channel partition x token free) so the recurrence layout is
    native; bf16 weights enable fast-weight-load and halve DMA traffic,
  - normalized gates f' = sig(f)/(sig(f)+sig(i)) via ACT sigmoids + one
    DVE reciprocal; the cheap elementwise products are offloaded to the
    otherwise-idle GPSIMD engine so the DVE chain stays under the PE time,
  - g = max(z + 0.5, sigmoid(z)) with z = cell + b the biased
    pre-activation (single ACT identity), fused via scalar_tensor_tensor,
  - the recurrence c_t = f'_t c_{t-1} + i'_t g_t with the DVE
    tensor_tensor_scan instruction in linear space (values bounded in
    [~4.5e-5, 10.5], so no log-space machinery is needed),
  - h = sig(o) * c.
Between the two layers, channel-half pairs exchange h1 (bf16) via pairwise
AllGather collectives (one per 512-token block, overlapped with compute).

Self-contained: hardcodes shapes; only imports the system concourse repo.
"""
import sys

if '/opt/trn_rl_repo' not in sys.path:
    sys.path.insert(0, '/opt/trn_rl_repo')

import numpy as np

B, S, D = 4, 4096, 1024
NCORES = 8
HALF = D // 2           # channels per core: 512
NCHUNK = HALF // 128    # 4 partition chunks of 128 channels
NKT = D // 128          # 8 contraction k-tiles
TBLK = 512              # token block
NBLK = S // TBLK        # 8 token blocks
GCH = 4 * HALF          # gate channels per core: 2048

_CACHE = {}

WSCALE = 64.0   # fp8 weight pre-scale (keeps W entries out of e4m3 denormals)


def _split_multi_waits(nc):
    """This walrus build rejects >1 sync wait per instruction. Hoist extra
    waits onto same-engine NoOps inserted just before; engine-queue program
    order makes this semantically identical."""
    from concourse import mybir
    n = 0
    for fn in nc.m.functions:
        for blk in fn.blocks:
            insts = list(blk.instructions)
            new = []
            changed = False
            for inst in insts:
                si = inst.sync_info
                ow = list(si.on_wait) if si is not None and si.on_wait else []
                if len(ow) > 1:
                    changed = True
                    for w in ow[:-1]:
                        n += 1
                        nop = mybir.InstNoOp(name=f"I-wsplit-{n}", ins=[], outs=[])
                        nop.engine = inst.engine
                        nop.sync_info = mybir.SyncInfo(on_wait=[w], on_update=[])
                        new.append(nop)
                    si.on_wait = [ow[-1]]
                new.append(inst)
            if changed:
                blk.instructions = new
    return n


def _build_nc(mm_mode="bf16", sim_local=False):
    import concourse.bass as bass
    import concourse.mybir as mybir
    import concourse.tile as tile

    f32 = mybir.dt.float32
    fmm = {"f32r": mybir.dt.float32r, "f32": f32,
           "bf16": mybir.dt.bfloat16, "fp8": mybir.dt.float8e4}[mm_mode]
    fh1 = {"f32r": f32, "f32": f32,
           "bf16": mybir.dt.bfloat16, "fp8": mybir.dt.float8e4}[mm_mode]
    is_fp8 = mm_mode == "fp8"
    gsc = 1.0 / WSCALE if is_fp8 else 1.0   # undo weight pre-scale at ACT
    AF = mybir.ActivationFunctionType
    ALU = mybir.AluOpType
    DR = mybir.MatmulPerfMode.DoubleRow

    nc = bass.Bass("TRN2", target_bir_lowering=False, debug=False,
                   num_devices=NCORES)

    xT_d = nc.dram_tensor("xT", [D, S], fmm, kind="ExternalInput").ap()
    w_d = [nc.dram_tensor(f"w{l}t", [D, GCH], fmm, kind="ExternalInput").ap()
           for l in range(2)]
    ba_d = [nc.dram_tensor(f"b{l}a", [128, 16], f32, kind="ExternalInput").ap()
            for l in range(2)]
    cp_d = [nc.dram_tensor(f"cp{l}", [128, 4], f32, kind="ExternalInput").ap()
            for l in range(2)]
    h2t_d = nc.dram_tensor("h2t", [HALF, S], f32, kind="ExternalOutput").ap()

    with tile.TileContext(nc) as tc:
        with tc.tile_pool(name="wp", bufs=2) as wp, \
             tc.tile_pool(name="xkp", bufs=2) as xkp, \
             tc.tile_pool(name="gp2", bufs=3) as gp2, \
             tc.tile_pool(name="gp3", bufs=3) as gp3, \
             tc.tile_pool(name="cp", bufs=1) as cpool, \
             tc.tile_pool(name="psum", bufs=8, space="PSUM") as psum, \
             tc.tile_pool(name="dstage", bufs=2, space="DRAM") as dstage, \
             tc.tile_pool(name="dfull", bufs=8, space="DRAM") as dfull:

            # h1 gathered blocks must persist through layer 2: 8 live tiles
            h1f = [dfull.tile([D, TBLK], fh1, tag="h1f", name=f"h1f{t}")
                   for t in range(NBLK)]

            def load_xk(l, t):
                """Emit the rhs k-tile DMA loads for (layer, block)."""
                tiles = []
                if is_fp8:
                    for k in range(NKT // 2):
                        xkt = xkp.tile([128, 2, TBLK], fmm, tag=f"xk{l}_{k}",
                                       name=f"xk{l}_{t}_{k}")
                        for i in range(2):
                            r0 = (2 * k + i) * 128
                            if l == 0:
                                src = xT_d[r0:r0 + 128,
                                           t * TBLK:(t + 1) * TBLK]
                            else:
                                src = h1f[t][r0:r0 + 128, :]
                            nc.sync.dma_start(xkt[:, i, :], src)
                        tiles.append(xkt)
                else:
                    for k in range(NKT):
                        xkt = xkp.tile([128, TBLK], fmm, tag=f"xk{l}_{k}",
                                       name=f"xk{l}_{t}_{k}")
                        if l == 0:
                            src = xT_d[k * 128:(k + 1) * 128,
                                       t * TBLK:(t + 1) * TBLK]
                        else:
                            src = h1f[t][k * 128:(k + 1) * 128, :]
                        nc.sync.dma_start(
                            xkt[:],
                            src if src.dtype == fmm else src.bitcast(fmm))
                        tiles.append(xkt)
                return tiles

            l2_pre = {}
            for l in range(2):
                w_ks = []
                if is_fp8:
                    for k in range(NKT // 2):
                        wk = wp.tile([128, 2, GCH], fmm, tag=f"Wk{k}",
                                     name=f"w{l}_{k}")
                        for i in range(2):
                            nc.sync.dma_start(
                                wk[:, i, :],
                                w_d[l][(2 * k + i) * 128:(2 * k + i + 1) * 128, :])
                        w_ks.append(wk)
                else:
                    for k in range(NKT):
                        wk = wp.tile([128, GCH], fmm, tag=f"Wk{k}", name=f"w{l}_{k}")
                        nc.sync.dma_start(wk[:], w_d[l][k * 128:(k + 1) * 128, :])
                        w_ks.append(wk)
                ba = cpool.tile([128, 16], f32, tag=f"ba{l}", name=f"ba{l}")
                nc.sync.dma_start(ba[:], ba_d[l][:])
                cp = cpool.tile([128, 4], f32, tag=f"cp{l}", name=f"cp{l}")
                nc.sync.dma_start(cp[:], cp_d[l][:])

                carry = [None] * NCHUNK
                for t in range(NBLK):
                    if l == 1 and t in l2_pre:
                        xk_ks = l2_pre.pop(t)
                    else:
                        xk_ks = load_xk(l, t)

                    if l == 0:
                        h1own = dstage.tile([HALF, TBLK], fh1, tag="h1own",
                                            name=f"h1own{t}")

                    for j in range(NCHUNK):
                        ps = {}
                        for qi, q in enumerate(("i", "f", "o", "cell")):
                            ct = qi * NCHUNK + j
                            p = psum.tile([128, TBLK], f32, tag="ps",
                                          name=f"ps_{q}{l}_{t}_{j}")
                            if is_fp8:
                                for k in range(NKT // 2):
                                    nc.tensor.matmul(
                                        p[:],
                                        w_ks[k][:, :, ct * 128:(ct + 1) * 128],
                                        xk_ks[k][:],
                                        start=(k == 0), stop=(k == NKT // 2 - 1),
                                        perf_mode=DR)
                            else:
                                for k in range(NKT):
                                    nc.tensor.matmul(
                                        p[:],
                                        w_ks[k][:, ct * 128:(ct + 1) * 128],
                                        xk_ks[k][:],
                                        start=(k == 0), stop=(k == NKT - 1))
                            ps[q] = p

                        sf = gp2.tile([128, TBLK], f32, tag="sf", name=f"sf{l}{t}{j}")
                        nc.scalar.activation(sf[:], ps["f"][:], AF.Sigmoid,
                                             bias=ba[:, 4 + j:5 + j], scale=gsc)
                        si = gp2.tile([128, TBLK], f32, tag="si", name=f"si{l}{t}{j}")
                        nc.scalar.activation(si[:], ps["i"][:], AF.Sigmoid,
                                             bias=ba[:, j:j + 1], scale=gsc)
                        # z = cell + b_cell (pre-activation); sg = sig(z);
                        # g = max(z + 0.5, sg)  == reference's g(z)
                        z = gp2.tile([128, TBLK], f32, tag="z", name=f"z{l}{t}{j}")
                        nc.scalar.activation(z[:], ps["cell"][:], AF.Identity,
                                             bias=ba[:, 12 + j:13 + j], scale=gsc)
                        sg = gp2.tile([128, TBLK], f32, tag="sg", name=f"sg{l}{t}{j}")
                        nc.scalar.activation(sg[:], z[:], AF.Sigmoid)
                        so = gp2.tile([128, TBLK], f32, tag="so", name=f"so{l}{t}{j}")
                        nc.scalar.activation(so[:], ps["o"][:], AF.Sigmoid,
                                             bias=ba[:, 8 + j:9 + j], scale=gsc)

                        # g first in the DVE queue: it must not serialize
                        # behind the ~3.3us reciprocal (bt -> scan needs it)
                        g = gp2.tile([128, TBLK], f32, tag="g", name=f"g{l}{t}{j}")
                        nc.vector.scalar_tensor_tensor(
                            g[:], z[:], 0.5, sg[:], ALU.add, ALU.max)
                        # cheap elementwise products go to the otherwise-idle
                        # GPSIMD engine
                        ssum = gp2.tile([128, TBLK], f32, tag="ssum",
                                        name=f"ss{l}{t}{j}")
                        nc.gpsimd.tensor_tensor(ssum[:], sf[:], si[:], ALU.add)
                        r = gp2.tile([128, TBLK], f32, tag="r", name=f"r{l}{t}{j}")
                        nc.vector.reciprocal(r[:], ssum[:])
                        a = gp3.tile([128, TBLK], f32, tag="a", name=f"a{l}{t}{j}")
                        nc.gpsimd.tensor_tensor(a[:], sf[:], r[:], ALU.mult)
                        ipr = gp2.tile([128, TBLK], f32, tag="ipr",
                                       name=f"ip{l}{t}{j}")
                        nc.gpsimd.tensor_tensor(ipr[:], si[:], r[:], ALU.mult)
                        bt = gp3.tile([128, TBLK], f32, tag="bt", name=f"bt{l}{t}{j}")
                        nc.gpsimd.tensor_tensor(bt[:], ipr[:], g[:], ALU.mult)
                        c = gp3.tile([128, TBLK], f32, tag=f"c{j}", name=f"c{l}{t}{j}")
                        init = cp[:, j:j + 1] if t == 0 else carry[j]
                        nc.vector.tensor_tensor_scan(c[:], a[:], bt[:], init,
                                                     ALU.mult, ALU.add)
                        carry[j] = c[:, TBLK - 1:TBLK]
                        hdt = fh1 if l == 0 else f32
                        h = gp3.tile([128, TBLK], hdt, tag=f"h{l}", name=f"h{l}{t}{j}")
                        nc.vector.tensor_tensor(h[:], so[:], c[:], ALU.mult)

                        if l == 0:
                            nc.sync.dma_start(
                                h1own[j * 128:(j + 1) * 128, :], h[:])
                        else:
                            nc.sync.dma_start(
                                h2t_d[j * 128:(j + 1) * 128,
                                      t * TBLK:(t + 1) * TBLK], h[:])

                    if l == 0:
                        if sim_local:
                            # collective-free stand-in with the same data
                            # volume, for single-core TimelineSim modeling
                            nc.sync.dma_start(h1f[t][0:HALF, :], h1own[:])
                            nc.sync.dma_start(h1f[t][HALF:D, :], h1own[:])
                        else:
                            nc.gpsimd.collective_compute(
                                "AllGather", ALU.bypass,
                                replica_groups=[[0, 1], [2, 3], [4, 5], [6, 7]],
                                ins=[h1own.opt()],
                                outs=[h1f[t].opt()],
                            )
                        # prefetch layer-2's first two x-blocks while layer 1
                        # finishes, so the sync queue isn't gated on block 7's
                        # full scan chain at the transition
                        if t >= NBLK - 2:
                            l2_pre[t - (NBLK - 2)] = load_xk(1, t - (NBLK - 2))

    _split_multi_waits(nc)
    return nc


def _shard_inputs(x, W0, b0, W1, b1, c0_prev, c1_prev, mm_mode="bf16"):
    import ml_dtypes
    mmdt = {"f32r": np.float32, "f32": np.float32,
            "bf16": ml_dtypes.bfloat16,
            "fp8": ml_dtypes.float8_e4m3fn}[mm_mode]
    wmul = WSCALE if mm_mode == "fp8" else 1.0
    x = np.asarray(x, dtype=np.float32)
    in_maps = []
    xT = [np.ascontiguousarray(x[b].T.astype(mmdt)) for b in range(B)]
    per_layer = []
    for (W, bb) in ((W0, b0), (W1, b1)):
        W = np.asarray(W, dtype=np.float32)
        bb = np.asarray(bb, dtype=np.float32)
        halves = []
        for h in range(2):
            rows = np.concatenate(
                [q * D + h * HALF + np.arange(HALF) for q in range(4)])
            wt = np.ascontiguousarray(
                (W[rows, :].T * np.float32(wmul)).astype(mmdt))  # (D, GCH)
            ba = np.ascontiguousarray(bb[rows].reshape(16, 128).T)  # (128,16)
            halves.append((wt, ba))
        per_layer.append(halves)
    cps = []
    for cprev in (c0_prev, c1_prev):
        cprev = np.asarray(cprev, dtype=np.float32)
        halves = []
        for b in range(B):
            row = []
            for h in range(2):
                seg = cprev[b, 0, h * HALF:(h + 1) * HALF]
                row.append(np.ascontiguousarray(seg.reshape(4, 128).T))
            halves.append(row)
        cps.append(halves)
    for k in range(NCORES):
        b, h = k // 2, k % 2
        m = {"xT": xT[b]}
        for l in range(2):
            wt, ba = per_layer[l][h]
            m[f"w{l}t"] = wt
            m[f"b{l}a"] = ba
            m[f"cp{l}"] = cps[l][b][h]
        in_maps.append(m)
    return in_maps


import os
MM_MODE = os.environ.get("MINLSTM_MM_MODE", "bf16")


def _get_nc():
    if "nc" not in _CACHE:
        _CACHE["nc"] = _build_nc(mm_mode=MM_MODE)
    return _CACHE["nc"]


def kernel(x, W0, b0, W1, b1, c0_prev, c1_prev):
    from concourse.bass_utils import run_bass_kernel_spmd

    nc = _get_nc()
    in_maps = _shard_inputs(x, W0, b0, W1, b1, c0_prev, c1_prev, MM_MODE)
    res = run_bass_kernel_spmd(nc, in_maps, list(range(NCORES)))
    out = np.empty((B, S, D), dtype=np.float32)
    for k in range(NCORES):
        b, h = k // 2, k % 2
        out[b, :, h * HALF:(h + 1) * HALF] = res.results[k]["h2t"].T
    return out


# revision 21
# speedup vs baseline: 1.0666x; 1.0440x over previous
"""minLSTM (2-layer, B=4, S=4096, D=1024) on 8 Trainium2 NeuronCores.

Sharding: core k -> (batch b = k//2, channel half h = k%2).
Each core computes all 4096 timesteps for its batch and its 512 channels:
  - gates via PE matmuls in bf16 (lhsT = W^T shard, rhs = x^T), laid out
    (gate-channel partition x token free) so the recurrence layout is
    native; bf16 weights enable fast-weight-load and halve DMA traffic,
  - normalized gates f' = sig(f)/(sig(f)+sig(i)) via ACT sigmoids + one
    DVE reciprocal; the cheap elementwise products are offloaded to the
    otherwise-idle GPSIMD engine so the DVE chain stays under the PE time,
  - g = max(z + 0.5, sigmoid(z)) with z = cell + b the biased
    pre-activation (single ACT identity), fused via scalar_tensor_tensor,
  - the recurrence c_t = f'_t c_{t-1} + i'_t g_t with the DVE
    tensor_tensor_scan instruction in linear space (values bounded in
    [~4.5e-5, 10.5], so no log-space machinery is needed),
  - h = sig(o) * c.
Between the two layers, channel-half pairs exchange h1 (bf16) via pairwise
AllGather collectives (one per 512-token block, overlapped with compute).

Self-contained: hardcodes shapes; only imports the system concourse repo.
"""
import sys

if '/opt/trn_rl_repo' not in sys.path:
    sys.path.insert(0, '/opt/trn_rl_repo')

import numpy as np

B, S, D = 4, 4096, 1024
NCORES = 8
HALF = D // 2           # channels per core: 512
NCHUNK = HALF // 128    # 4 partition chunks of 128 channels
NKT = D // 128          # 8 contraction k-tiles
TBLK = 512              # token block
NBLK = S // TBLK        # 8 token blocks
GCH = 4 * HALF          # gate channels per core: 2048

_CACHE = {}

WSCALE = 64.0   # fp8 weight pre-scale (keeps W entries out of e4m3 denormals)


def _split_multi_waits(nc):
    """This walrus build rejects >1 sync wait per instruction. Hoist extra
    waits onto same-engine NoOps inserted just before; engine-queue program
    order makes this semantically identical."""
    from concourse import mybir
    n = 0
    for fn in nc.m.functions:
        for blk in fn.blocks:
            insts = list(blk.instructions)
            new = []
            changed = False
            for inst in insts:
                si = inst.sync_info
                ow = list(si.on_wait) if si is not None and si.on_wait else []
                if len(ow) > 1:
                    changed = True
                    for w in ow[:-1]:
                        n += 1
                        nop = mybir.InstNoOp(name=f"I-wsplit-{n}", ins=[], outs=[])
                        nop.engine = inst.engine
                        nop.sync_info = mybir.SyncInfo(on_wait=[w], on_update=[])
                        new.append(nop)
                    si.on_wait = [ow[-1]]
                new.append(inst)
            if changed:
                blk.instructions = new
    return n


def _build_nc(mm_mode="bf16", sim_local=False):
    import concourse.bass as bass
    import concourse.mybir as mybir
    import concourse.tile as tile

    f32 = mybir.dt.float32
    fmm = {"f32r": mybir.dt.float32r, "f32": f32,
           "bf16": mybir.dt.bfloat16, "fp8": mybir.dt.float8e4}[mm_mode]
    fh1 = {"f32r": f32, "f32": f32,
           "bf16": mybir.dt.bfloat16, "fp8": mybir.dt.float8e4}[mm_mode]
    is_fp8 = mm_mode == "fp8"
    gsc = 1.0 / WSCALE if is_fp8 else 1.0   # undo weight pre-scale at ACT
    AF = mybir.ActivationFunctionType
    ALU = mybir.AluOpType
    DR = mybir.MatmulPerfMode.DoubleRow

    nc = bass.Bass("TRN2", target_bir_lowering=False, debug=False,
                   num_devices=NCORES)

    xT_d = nc.dram_tensor("xT", [D, S], fmm, kind="ExternalInput").ap()
    w_d = [nc.dram_tensor(f"w{l}t", [D, GCH], fmm, kind="ExternalInput").ap()
           for l in range(2)]
    ba_d = [nc.dram_tensor(f"b{l}a", [128, 16], f32, kind="ExternalInput").ap()
            for l in range(2)]
    cp_d = [nc.dram_tensor(f"cp{l}", [128, 4], f32, kind="ExternalInput").ap()
            for l in range(2)]
    h2t_d = nc.dram_tensor("h2t", [HALF, S], f32, kind="ExternalOutput").ap()

    with tile.TileContext(nc) as tc:
        with tc.tile_pool(name="wp", bufs=2) as wp, \
             tc.tile_pool(name="xkp", bufs=2) as xkp, \
             tc.tile_pool(name="gp2", bufs=4) as gp2, \
             tc.tile_pool(name="gp2b", bufs=2) as gp2b, \
             tc.tile_pool(name="gp3", bufs=3) as gp3, \
             tc.tile_pool(name="cp", bufs=1) as cpool, \
             tc.tile_pool(name="psum", bufs=8, space="PSUM") as psum, \
             tc.tile_pool(name="dstage", bufs=2, space="DRAM") as dstage, \
             tc.tile_pool(name="dfull", bufs=8, space="DRAM") as dfull:

            # h1 gathered blocks must persist through layer 2: 8 live tiles
            h1f = [dfull.tile([D, TBLK], fh1, tag="h1f", name=f"h1f{t}")
                   for t in range(NBLK)]

            def load_xk(l, t):
                """Emit the rhs k-tile DMA loads for (layer, block)."""
                tiles = []
                if is_fp8:
                    for k in range(NKT // 2):
                        xkt = xkp.tile([128, 2, TBLK], fmm, tag=f"xk{l}_{k}",
                                       name=f"xk{l}_{t}_{k}")
                        for i in range(2):
                            r0 = (2 * k + i) * 128
                            if l == 0:
                                src = xT_d[r0:r0 + 128,
                                           t * TBLK:(t + 1) * TBLK]
                            else:
                                src = h1f[t][r0:r0 + 128, :]
                            nc.sync.dma_start(xkt[:, i, :], src)
                        tiles.append(xkt)
                else:
                    for k in range(NKT):
                        xkt = xkp.tile([128, TBLK], fmm, tag=f"xk{l}_{k}",
                                       name=f"xk{l}_{t}_{k}")
                        if l == 0:
                            src = xT_d[k * 128:(k + 1) * 128,
                                       t * TBLK:(t + 1) * TBLK]
                        else:
                            src = h1f[t][k * 128:(k + 1) * 128, :]
                        nc.sync.dma_start(
                            xkt[:],
                            src if src.dtype == fmm else src.bitcast(fmm))
                        tiles.append(xkt)
                return tiles

            l2_pre = {}
            for l in range(2):
                w_ks = []
                if is_fp8:
                    for k in range(NKT // 2):
                        wk = wp.tile([128, 2, GCH], fmm, tag=f"Wk{k}",
                                     name=f"w{l}_{k}")
                        for i in range(2):
                            nc.sync.dma_start(
                                wk[:, i, :],
                                w_d[l][(2 * k + i) * 128:(2 * k + i + 1) * 128, :])
                        w_ks.append(wk)
                else:
                    for k in range(NKT):
                        wk = wp.tile([128, GCH], fmm, tag=f"Wk{k}", name=f"w{l}_{k}")
                        nc.sync.dma_start(wk[:], w_d[l][k * 128:(k + 1) * 128, :])
                        w_ks.append(wk)
                ba = cpool.tile([128, 16], f32, tag=f"ba{l}", name=f"ba{l}")
                nc.sync.dma_start(ba[:], ba_d[l][:])
                cp = cpool.tile([128, 4], f32, tag=f"cp{l}", name=f"cp{l}")
                nc.sync.dma_start(cp[:], cp_d[l][:])

                carry = [None] * NCHUNK
                pending = [None]  # lagged stage-B args (software pipeline)

                def stage_b():
                    """Emit the scan + h + h-DMA for the previous chunk: one
                    chunk behind stage A so neither engine queue head-of-line
                    blocks on the other engine's in-flight work."""
                    if pending[0] is None:
                        return
                    (pt, pj, pa, pbt, pso, ph1own) = pending[0]
                    pending[0] = None
                    c = gp3.tile([128, TBLK], f32, tag=f"c{pj}",
                                 name=f"c{l}{pt}{pj}")
                    init = cp[:, pj:pj + 1] if pt == 0 else carry[pj]
                    nc.vector.tensor_tensor_scan(c[:], pa[:], pbt[:], init,
                                                 ALU.mult, ALU.add)
                    carry[pj] = c[:, TBLK - 1:TBLK]
                    hdt = fh1 if l == 0 else f32
                    h = gp3.tile([128, TBLK], hdt, tag=f"h{l}",
                                 name=f"h{l}{pt}{pj}")
                    if l == 0:
                        nc.vector.tensor_tensor(h[:], pso[:], c[:], ALU.mult)
                        nc.sync.dma_start(
                            ph1own[pj * 128:(pj + 1) * 128, :], h[:])
                    else:
                        nc.gpsimd.tensor_tensor(h[:], pso[:], c[:], ALU.mult)
                        nc.sync.dma_start(
                            h2t_d[pj * 128:(pj + 1) * 128,
                                  pt * TBLK:(pt + 1) * TBLK], h[:])
                    if l == 0 and pj == NCHUNK - 1:
                        # block pt's h1own is complete: exchange it
                        nc.gpsimd.collective_compute(
                            "AllGather", ALU.bypass,
                            replica_groups=[[0, 1], [2, 3], [4, 5], [6, 7]],
                            ins=[ph1own.opt()],
                            outs=[h1f[pt].opt()],
                        )
                        # prefetch layer-2's first two x-blocks while layer 1
                        # finishes, so the sync queue isn't gated on block 7's
                        # full scan chain at the transition
                        if pt >= NBLK - 2:
                            l2_pre[pt - (NBLK - 2)] = load_xk(1, pt - (NBLK - 2))

                for t in range(NBLK):
                    if l == 1 and t in l2_pre:
                        xk_ks = l2_pre.pop(t)
                    else:
                        xk_ks = load_xk(l, t)

                    if l == 0:
                        h1own = dstage.tile([HALF, TBLK], fh1, tag="h1own",
                                            name=f"h1own{t}")

                    for j in range(NCHUNK):
                        ps = {}
                        for qi, q in enumerate(("i", "f", "o", "cell")):
                            ct = qi * NCHUNK + j
                            p = psum.tile([128, TBLK], f32, tag="ps",
                                          name=f"ps_{q}{l}_{t}_{j}")
                            if is_fp8:
                                for k in range(NKT // 2):
                                    nc.tensor.matmul(
                                        p[:],
                                        w_ks[k][:, :, ct * 128:(ct + 1) * 128],
                                        xk_ks[k][:],
                                        start=(k == 0), stop=(k == NKT // 2 - 1),
                                        perf_mode=DR)
                            else:
                                for k in range(NKT):
                                    nc.tensor.matmul(
                                        p[:],
                                        w_ks[k][:, ct * 128:(ct + 1) * 128],
                                        xk_ks[k][:],
                                        start=(k == 0), stop=(k == NKT - 1))
                            ps[q] = p

                        sf = gp2.tile([128, TBLK], f32, tag="sf", name=f"sf{l}{t}{j}")
                        nc.scalar.activation(sf[:], ps["f"][:], AF.Sigmoid,
                                             bias=ba[:, 4 + j:5 + j], scale=gsc)
                        si = gp2.tile([128, TBLK], f32, tag="si", name=f"si{l}{t}{j}")
                        nc.scalar.activation(si[:], ps["i"][:], AF.Sigmoid,
                                             bias=ba[:, j:j + 1], scale=gsc)
                        # z = cell + b_cell (pre-activation); sg = sig(z);
                        # g = max(z + 0.5, sg)  == reference's g(z)
                        z = gp2b.tile([128, TBLK], f32, tag="z", name=f"z{l}{t}{j}")
                        nc.scalar.activation(z[:], ps["cell"][:], AF.Identity,
                                             bias=ba[:, 12 + j:13 + j], scale=gsc)
                        sg = gp2b.tile([128, TBLK], f32, tag="sg", name=f"sg{l}{t}{j}")
                        nc.scalar.activation(sg[:], z[:], AF.Sigmoid)
                        so = gp2.tile([128, TBLK], f32, tag="so", name=f"so{l}{t}{j}")
                        nc.scalar.activation(so[:], ps["o"][:], AF.Sigmoid,
                                             bias=ba[:, 8 + j:9 + j], scale=gsc)

                        # g first in the DVE queue: it must not serialize
                        # behind the ~3.3us reciprocal (bt -> scan needs it)
                        g = gp2.tile([128, TBLK], f32, tag="g", name=f"g{l}{t}{j}")
                        nc.vector.scalar_tensor_tensor(
                            g[:], z[:], 0.5, sg[:], ALU.add, ALU.max)
                        # ssum on the (otherwise idle) GPSIMD engine
                        ssum = gp2.tile([128, TBLK], f32, tag="ssum",
                                        name=f"ss{l}{t}{j}")
                        nc.gpsimd.tensor_tensor(ssum[:], sf[:], si[:], ALU.add)
                        r = gp2.tile([128, TBLK], f32, tag="r", name=f"r{l}{t}{j}")
                        nc.vector.reciprocal(r[:], ssum[:])
                        a = gp3.tile([128, TBLK], f32, tag="a", name=f"a{l}{t}{j}")
                        nc.vector.tensor_tensor(a[:], sf[:], r[:], ALU.mult)
                        ipr = gp2.tile([128, TBLK], f32, tag="ipr",
                                       name=f"ip{l}{t}{j}")
                        nc.vector.tensor_scalar(ipr[:], a[:], -1.0, 1.0,
                                                ALU.mult, ALU.add)
                        bt = gp3.tile([128, TBLK], f32, tag="bt", name=f"bt{l}{t}{j}")
                        nc.gpsimd.tensor_tensor(bt[:], ipr[:], g[:], ALU.mult)

                        # previous chunk's scan/h, then queue this chunk's
                        stage_b()
                        pending[0] = (t, j, a, bt, so,
                                      h1own if l == 0 else None)

                # layer flush: last chunk's scan/h (+ final AllGather for l=0)
                stage_b()

    _split_multi_waits(nc)
    return nc


def _shard_inputs(x, W0, b0, W1, b1, c0_prev, c1_prev, mm_mode="bf16"):
    import ml_dtypes
    mmdt = {"f32r": np.float32, "f32": np.float32,
            "bf16": ml_dtypes.bfloat16,
            "fp8": ml_dtypes.float8_e4m3fn}[mm_mode]
    wmul = WSCALE if mm_mode == "fp8" else 1.0
    x = np.asarray(x, dtype=np.float32)
    in_maps = []
    xT = [np.ascontiguousarray(x[b].T.astype(mmdt)) for b in range(B)]
    per_layer = []
    for (W, bb) in ((W0, b0), (W1, b1)):
        W = np.asarray(W, dtype=np.float32)
        bb = np.asarray(bb, dtype=np.float32)
        halves = []
        for h in range(2):
            rows = np.concatenate(
                [q * D + h * HALF + np.arange(HALF) for q in range(4)])
            wt = np.ascontiguousarray(
                (W[rows, :].T * np.float32(wmul)).astype(mmdt))  # (D, GCH)
            ba = np.ascontiguousarray(bb[rows].reshape(16, 128).T)  # (128,16)
            halves.append((wt, ba))
        per_layer.append(halves)
    cps = []
    for cprev in (c0_prev, c1_prev):
        cprev = np.asarray(cprev, dtype=np.float32)
        halves = []
        for b in range(B):
            row = []
            for h in range(2):
                seg = cprev[b, 0, h * HALF:(h + 1) * HALF]
                row.append(np.ascontiguousarray(seg.reshape(4, 128).T))
            halves.append(row)
        cps.append(halves)
    for k in range(NCORES):
        b, h = k // 2, k % 2
        m = {"xT": xT[b]}
        for l in range(2):
            wt, ba = per_layer[l][h]
            m[f"w{l}t"] = wt
            m[f"b{l}a"] = ba
            m[f"cp{l}"] = cps[l][b][h]
        in_maps.append(m)
    return in_maps


import os
MM_MODE = os.environ.get("MINLSTM_MM_MODE", "bf16")


def _get_nc():
    if "nc" not in _CACHE:
        _CACHE["nc"] = _build_nc(mm_mode=MM_MODE)
    return _CACHE["nc"]


def kernel(x, W0, b0, W1, b1, c0_prev, c1_prev):
    from concourse.bass_utils import run_bass_kernel_spmd

    nc = _get_nc()
    in_maps = _shard_inputs(x, W0, b0, W1, b1, c0_prev, c1_prev, MM_MODE)
    res = run_bass_kernel_spmd(nc, in_maps, list(range(NCORES)))
    out = np.empty((B, S, D), dtype=np.float32)
    for k in range(NCORES):
        b, h = k // 2, k % 2
        out[b, :, h * HALF:(h + 1) * HALF] = res.results[k]["h2t"].T
    return out
